# revision 3
# baseline (speedup 1.0000x reference)
"""ConvDU Trainium2 Bass kernel.

Reference computation (per batch):
    rows = fea[b]  # (c=256, h=96, w=96), conv = Conv1d(c->c, kw=9, pad=4) over w
    down: s[0] = row[0];  s[i] = relu(conv(s[i-1]) + bias) + row[i]
    up:   t[95] = s[95];  t[i] = relu(conv(t[i+1]) + bias) + s[i]
    out[b] = t

Strategy: data-parallel over batch n (8 batches -> 8 NeuronCores), zero
inter-core communication.  Per core, each conv step is 2 PSUM accumulation
groups (one per 128-channel output half): a K=1 bias matmul + 18 fp16
matmuls (9 taps x 2 input-channel halves), N=96 free dim, fp32 PSUM
accumulate.  The relu+residual is a fused DVE scalar_tensor_tensor
(out = max(psum,0) + row), written twice: once as fp32 into the in-place
SBUF state buffer, once as fp16 into the next step's padded rhs buffer.
The full (c,h,w) state stays SBUF-resident; fea is DMA'd in once and
output rows are DMA'd out as the up-pass produces them.

fp16 matmul operand rounding gives ~3.6e-4 final relative L2 error
(measured vs fp32 reference through the full 190-step recurrence);
accumulation is exact fp32 on the PE.
"""

import numpy as np

C = 256          # channels
H = 96           # rows (recurrence length)
W = 96           # row width
KW = 9           # conv kernel width
PADW = W + 8     # padded fp16 row width (4 zeros each side)
N_CORES = 8
HW_FLAT = H * W

_CACHE = {}


def _build_nc(h=H):
    import concourse.mybir as mybir
    import concourse.tile as tile
    from concourse import bacc

    hw_flat = h * W
    nc = bacc.Bacc(trn_type="TRN2")
    fea_d = nc.dram_tensor("fea", (C, hw_flat), mybir.dt.float32, kind="ExternalInput")
    w_d = nc.dram_tensor("w", (C, 2 * KW * 128), mybir.dt.float16, kind="ExternalInput")
    b_d = nc.dram_tensor("bias", (1, C), mybir.dt.float16, kind="ExternalInput")
    out_d = nc.dram_tensor("out", (C, hw_flat), mybir.dt.float32, kind="ExternalOutput")

    f32 = mybir.dt.float32
    f16 = mybir.dt.float16
    AOT = mybir.AluOpType

    with tile.TileContext(nc) as tc:
        with tc.tile_pool(name="state", bufs=1) as state_pool, \
             tc.tile_pool(name="psum", bufs=8, space="PSUM") as psum_pool:
            # Persistent SBUF state
            X = [state_pool.tile([128, hw_flat], f32, tag=f"X{kh}", name=f"X{kh}") for kh in range(2)]
            Wsb = [state_pool.tile([128, KW * 2 * 128], f16, tag=f"W{kh}", name=f"Wsb{kh}") for kh in range(2)]
            bsb = state_pool.tile([1, C], f16, tag="bsb")
            ones = state_pool.tile([1, W], f16, tag="ones")
            # ping/pong padded fp16 rhs rows; both kh halves side by side
            cur = [state_pool.tile([128, 2 * PADW], f16, tag=f"cur{p}", name=f"cur{p}") for p in range(2)]

            # Loads
            n_chunks = 8 if h >= 8 else 1
            rows_per_chunk = h // n_chunks
            for kh in range(2):
                nc.sync.dma_start(Wsb[kh][:], w_d[kh * 128:(kh + 1) * 128, :])
                for ch in range(n_chunks):
                    c0 = ch * rows_per_chunk * W
                    c1 = hw_flat if ch == n_chunks - 1 else (ch + 1) * rows_per_chunk * W
                    nc.sync.dma_start(X[kh][:, c0:c1], fea_d[kh * 128:(kh + 1) * 128, c0:c1])
            nc.sync.dma_start(bsb[:], b_d[:])
            nc.gpsimd.memset(ones[:], 1.0)
            nc.gpsimd.memset(cur[0][:], 0.0)
            nc.gpsimd.memset(cur[1][:], 0.0)

            # s[0] = row 0: cast into cur[0]
            for kh in range(2):
                nc.vector.tensor_copy(cur[0][:, kh * PADW + 4: kh * PADW + 4 + W],
                                      X[kh][:, 0:W])

            def conv_step(prev, nxt, res_row, emit_out, write_next):
                """new = relu(conv(cur[prev]) + b) + X[res_row];
                X[res_row] <- new (fp32), cur[nxt] <- new (fp16)."""
                r0 = res_row * W
                for m in range(2):
                    ps = psum_pool.tile([128, W], f32, tag="ps", name="ps")
                    # bias via K=1 matmul (group opener, no data deps)
                    nc.tensor.matmul(ps[:], bsb[:, m * 128:(m + 1) * 128],
                                     ones[:, 0:W], start=True, stop=False)
                    for kh in range(2):
                        base = kh * PADW
                        for tap in range(KW):
                            nc.tensor.matmul(
                                ps[:],
                                Wsb[kh][:, (m * KW + tap) * 128:(m * KW + tap + 1) * 128],
                                cur[prev][:, base + tap: base + tap + W],
                                start=False, stop=(kh == 1 and tap == KW - 1))
                    xrow = X[m][:, r0:r0 + W]
                    if write_next:
                        # fp16 copy first: it must read the OLD X row
                        nc.vector.scalar_tensor_tensor(
                            cur[nxt][:, m * PADW + 4: m * PADW + 4 + W],
                            ps[:], 0.0, xrow, op0=AOT.max, op1=AOT.add)
                    nc.vector.scalar_tensor_tensor(
                        xrow, ps[:], 0.0, xrow, op0=AOT.max, op1=AOT.add)
                    if emit_out:
                        nc.scalar.dma_start(out_d[m * 128:(m + 1) * 128, r0:r0 + W], xrow)

            # down pass: rows 1..h-1 (step g has prev=g%2? maintain toggle)
            g = 0
            for i in range(1, h):
                conv_step(g % 2, (g + 1) % 2, i, emit_out=False, write_next=True)
                g += 1
            # row h-1 now holds s_down[h-1] == t[h-1]: emit it
            for m in range(2):
                r0 = (h - 1) * W
                nc.scalar.dma_start(out_d[m * 128:(m + 1) * 128, r0:r0 + W],
                                    X[m][:, r0:r0 + W])
            # up pass: rows h-2 .. 0
            for i in range(h - 2, -1, -1):
                conv_step(g % 2, (g + 1) % 2, i, emit_out=True, write_next=(i != 0))
                g += 1

    nc.compile()
    return nc


def _pack_weights(W_arr, b_arr):
    # lhsT for (kh, m, tap): [ci 128, co 128] = W[m*128:(m+1)*128, kh*128:(kh+1)*128, tap].T
    Wt = np.ascontiguousarray(W_arr.transpose(1, 0, 2))  # (ci, co, tap)
    packed = np.empty((C, 2 * KW * 128), dtype=np.float16)
    for kh in range(2):
        sl = Wt[kh * 128:(kh + 1) * 128]           # (128, 256, 9)
        arr = sl.reshape(128, 2, 128, KW)          # (ci, m, co, tap)
        arr = arr.transpose(0, 1, 3, 2)            # (ci, m, tap, co)
        packed[kh * 128:(kh + 1) * 128] = arr.reshape(128, 2 * KW * 128).astype(np.float16)
    b16 = b_arr.astype(np.float16).reshape(1, C)
    return packed, b16


def kernel(fea, W, b):
    from concourse import bass_utils

    fea = np.asarray(fea)
    W_arr = np.asarray(W)
    b_arr = np.asarray(b)
    n = fea.shape[0]
    assert fea.shape == (n, C, H, 96), fea.shape

    if "nc" not in _CACHE:
        _CACHE["nc"] = _build_nc()
    nc = _CACHE["nc"]

    packed, b16 = _pack_weights(W_arr, b_arr)
    in_maps = [{
        "fea": np.ascontiguousarray(fea[c].reshape(C, HW_FLAT), dtype=np.float32),
        "w": packed,
        "bias": b16,
    } for c in range(n)]

    res = bass_utils.run_bass_kernel_spmd(nc, in_maps, core_ids=list(range(n)))
    out = np.stack([r["out"].reshape(C, H, 96) for r in res.results], axis=0)
    return out.astype(np.float32)
